# revision 23
# baseline (speedup 1.0000x reference)
"""Bass/Tile TRN2 kernel for nn_CenterAlignedTripletLoss (8-core SPMD).

Sharding:
  Phase 1 (dist/mining scores): feats sharded along the FEATURE axis
    (8192 -> 1024 per core).  Each core computes a partial mining score
    s[n, m] = ||f_m||^2 - 2 c_n . f_m   over its feature slice
    (row-constant ||c_n||^2 omitted: argmax/argmin per row unaffected),
    accumulated in PSUM via matmuls.  A ReduceScatter(add) over the
    96-center axis gives each core the full scores for its 12 centers.
  Centers: computed per-core in feature-major layout from the same
    feats tiles, transposed to center-major [96, 1024-slice] on PE, and
    exchanged with an AllToAll so core c ends with full-feature centers
    for its 12-center block.
  Phase 2 (mining + local-distance DP): each core mines hardest
    positive/negative for its 12 centers, indirect-DMA gathers the 24
    winning local_features rows, computes the 8x8 stripe distance
    matrices via DVE diff + ACT square-accumulate, applies
    sqrt -> tanh(d/2), runs the shortest-path DP on anti-diagonals, and
    reduces to a per-core partial sum of relu(ap - an + margin).
  Host: sums the 8 partial scalars / 96.
"""

import os
import numpy as np
from contextlib import ExitStack

import concourse.bass as bass
import concourse.bacc as bacc
import concourse.tile as tile
from concourse import mybir
from concourse import bass_utils

F32 = mybir.dt.float32
U32 = mybir.dt.uint32
AF = mybir.ActivationFunctionType
ALU = mybir.AluOpType

NCORES = 8
M = 1536          # samples
D = 8192          # feature dim
DPC = D // NCORES # 1024 features per core
N = 96            # centers
NB = N // NCORES  # 12 centers per core
K = 16            # samples per chunk
S = 8             # stripes
DL = 1024         # local feature dim per stripe
MARGIN = 0.3
BIGM = 1.0e9      # mining mask penalty
BIGDP = 1.0e6     # DP pad value
RG = [list(range(NCORES))]


def build_body(tc, out, ins):
    nc = tc.nc
    feats = ins["feats_sl"]     # [M, DPC] this core's feature slice
    lf = ins["lf"]              # [M, D] full flattened local_features
    lab = ins["lab"]            # [1, M] labels as f32
    anc = ins["anc"]            # [NB, 1] this core's anchor labels (f32)
    wsel = ins["wsel"]          # [64, NB] +I (rows 0-11) / -I (rows 32-43)
    ident = ins["ident"]        # [128, 128] identity
    esel = ins["esel"]          # [96, 96] fold selection

    featsT = feats.rearrange("m d -> d m")  # [DPC, M] transposed view

    with ExitStack() as ctx:
        const = ctx.enter_context(tc.tile_pool(name="const", bufs=1))
        mine = ctx.enter_context(tc.tile_pool(name="mine", bufs=4))
        pers = ctx.enter_context(tc.tile_pool(name="pers", bufs=1))
        dram = ctx.enter_context(tc.tile_pool(name="dram", bufs=1, space="DRAM"))

        # ---- constants ----
        ident_sb = const.tile([128, 128], F32)
        nc.sync.dma_start(ident_sb, ident)
        anc_sb = const.tile([NB, 1], F32)
        nc.sync.dma_start(anc_sb, anc)
        wsel_sb = const.tile([64, NB], F32)
        nc.sync.dma_start(wsel_sb, wsel)
        lab_sb = const.tile([1, M], F32)
        nc.sync.dma_start(lab_sb, lab)
        esel_sb = const.tile([N, N], F32)
        nc.sync.dma_start(esel_sb, esel)
        ones_k = const.tile([128, N], F32)
        nc.vector.memset(ones_k, 1.0)
        ones_r = const.tile([1, NB], F32)
        nc.vector.memset(ones_r, 1.0)
        ones_c = const.tile([NB, 1], F32)
        nc.vector.memset(ones_c, 1.0)
        marg = const.tile([NB, 1], F32)
        nc.vector.memset(marg, MARGIN)

        ctr_cm = pers.tile([N, DPC], F32)   # center-major centers slice
        # s_sb and x_all are time-disjoint: share one 32KB/partition slot
        s_sb = pers.tile([N, M], F32)   # partial mining scores

        # DRAM bounce buffers for collectives
        rs_in = dram.tile([N, M], F32)
        rs_out = dram.tile([NB, M], F32)
        a2a_in = dram.tile([N, DPC], F32)
        a2a_out = dram.tile([N, DPC], F32)

        # ---- phase 1: partial scores + centers over 8 feature k-tiles ----
        with tc.tile_pool(name="psum1", bufs=1, space="PSUM") as psum1, \
             tc.tile_pool(name="psumt", bufs=2, space="PSUM") as psumt, \
             tc.tile_pool(name="kio", bufs=2) as kio, \
             tc.tile_pool(name="workp1", bufs=2) as work:
            s_ps = [psum1.tile([N, 512], F32, name=f"s_ps{b}") for b in range(3)]
            for kd in range(8):
                ft = kio.tile([128, M], F32, tag="ft")
                nc.sync.dma_start(ft, featsT[kd * 128:(kd + 1) * 128, :])
                csum = work.tile([128, N], F32, tag="csum")
                nc.vector.reduce_sum(
                    out=csum,
                    in_=ft.rearrange("p (n k) -> p n k", k=K),
                    axis=mybir.AxisListType.X,
                )
                ctrm2 = work.tile([128, N], F32, tag="ctrm2")
                nc.vector.tensor_scalar_mul(ctrm2, csum, -2.0 / K)
                sq = kio.tile([128, M], F32, tag="sq")
                nc.scalar.activation(sq, ft, AF.Square)
                for b in range(3):
                    nc.tensor.matmul(
                        s_ps[b], lhsT=ctrm2, rhs=ft[:, b * 512:(b + 1) * 512],
                        start=(kd == 0), stop=False,
                    )
                    nc.tensor.matmul(
                        s_ps[b], lhsT=ones_k, rhs=sq[:, b * 512:(b + 1) * 512],
                        start=False, stop=(kd == 7),
                    )
                # true centers (scaled 1/16), transposed to center-major
                tp = psumt.tile([N, 128], F32, tag="tp")
                nc.tensor.transpose(tp, csum, ident_sb)
                nc.scalar.activation(
                    ctr_cm[:, kd * 128:(kd + 1) * 128], tp, AF.Copy, scale=1.0 / K
                )
            for b in range(3):
                nc.vector.tensor_copy(s_sb[:, b * 512:(b + 1) * 512], s_ps[b])

        nc.sync.dma_start(rs_in, s_sb)
        nc.sync.dma_start(a2a_in, ctr_cm)
        nc.gpsimd.collective_compute(
            "ReduceScatter", ALU.add, replica_groups=RG,
            ins=[rs_in.opt()], outs=[rs_out.opt()],
        )
        nc.gpsimd.collective_compute(
            "AllToAll", ALU.bypass, replica_groups=RG,
            ins=[a2a_in.opt()], outs=[a2a_out.opt()],
        )

        # ---- X_all prefetch: [96 (b,j-rep), 8192 (i,d)] from a2a_out ----
        # a2a_out row (i*NB + b) holds center b's features [i*1024, (i+1)*1024)
        x_all = pers.tile([N, D], F32)
        x_v = a2a_out.rearrange("(i b) d -> b i d", b=NB)  # [12, 8, 1024]
        for b in range(NB):
            # center b's full row, replicated over the 8 j-partitions
            nc.sync.dma_start(
                x_all[b * S:(b + 1) * S, :],
                x_v[b:b + 1, :, :].to_broadcast([S, S, DPC]),
            )

        # ---- mining on this core's 12 score rows ----
        s12 = mine.tile([NB, M], F32, tag="mine", name="s12")
        nc.sync.dma_start(s12, rs_out)
        diff = mine.tile([NB, M], F32, tag="mine", name="diff")
        m01 = mine.tile([NB, M], F32, tag="mine", name="m01")
        m01c = mine.tile([NB, M], F32, tag="mine", name="m01c")
        pos = mine.tile([NB, M], F32, tag="mine", name="pos")
        neg = mine.tile([NB, M], F32, tag="mine", name="neg")
        with tc.tile_pool(name="psum2", bufs=1, space="PSUM") as psum2:
            lbc = psum2.tile([NB, M], F32)
            for b in range(3):
                nc.tensor.matmul(
                    lbc[:, b * 512:(b + 1) * 512], lhsT=ones_r,
                    rhs=lab_sb[:, b * 512:(b + 1) * 512], start=True, stop=True,
                )
            # diff = L[m] - A[b] per partition
            nc.vector.tensor_scalar(
                out=diff, in0=lbc, scalar1=anc_sb, scalar2=None, op0=ALU.subtract
            )
        nc.vector.tensor_tensor(out=m01, in0=diff, in1=diff, op=ALU.mult)
        nc.vector.tensor_scalar_min(m01c, m01, 1.0)  # 1 iff labels differ
        # pos = s - BIGM*m01c ; neg = BIGM*m01c - BIGM - s
        nc.vector.tensor_scalar(
            out=pos, in0=m01c, scalar1=-BIGM, scalar2=None, op0=ALU.mult
        )
        nc.vector.tensor_tensor(out=pos, in0=pos, in1=s12, op=ALU.add)
        nc.vector.tensor_scalar(
            out=neg, in0=m01c, scalar1=BIGM, scalar2=-BIGM, op0=ALU.mult, op1=ALU.add
        )
        nc.vector.tensor_tensor(out=neg, in0=neg, in1=s12, op=ALU.subtract)

        pmax = pers.tile([NB, 8], F32, tag="pmax")
        pidx = pers.tile([NB, 8], U32)
        nmax = pers.tile([NB, 8], F32, tag="nmax")
        nidx = pers.tile([NB, 8], U32)
        nc.vector.max(pmax, pos)
        nc.vector.max_index(pidx, pmax, pos)
        nc.vector.max(nmax, neg)
        nc.vector.max_index(nidx, nmax, neg)

        # ---- phase 2: gather winners + local distances ----
        # tanh'd dists, padded 9x9 grid; p-set at partitions 0-11, n-set at
        # 32-43 (compute ops need 32-aligned start partitions)
        dD = pers.tile([64, 81], F32)
        nc.vector.memset(dD, 0.0)
        with tc.tile_pool(name="yrawp", bufs=1) as yrawp, \
             tc.tile_pool(name="deip", bufs=1) as deip, \
             tc.tile_pool(name="workp2", bufs=2) as work2:
            for iset, idx in enumerate((pidx, nidx)):
                raw = yrawp.tile([NB, D], F32, tag="yraw", name=f"yraw{iset}")
                nc.gpsimd.indirect_dma_start(
                    out=raw, out_offset=None, in_=lf,
                    in_offset=bass.IndirectOffsetOnAxis(ap=idx[:, :1], axis=0),
                )
                # de-interleave [12, (d, j)] -> [12, (j, d)]
                dei = deip.tile([NB, D], F32, tag="dei", name=f"dei{iset}")
                nc.vector.tensor_copy(
                    out=dei.rearrange("p (j d) -> p j d", j=S),
                    in_=raw.rearrange("p (d j) -> p j d", j=S),
                )
                # fold to 96 partitions: [12, 8192] -> [96 (b,j), 1024 (d)]
                y96 = work2.tile([N, DL], F32, tag="y96")
                nc.sync.dma_start(y96, dei)
                d2t = work2.tile([N, S], F32, tag="d2t")
                for i in range(S):
                    dtmp = work2.tile([N, DL], F32, tag="dtmp")
                    nc.vector.tensor_tensor(
                        out=dtmp, in0=y96, in1=x_all[:, i * DL:(i + 1) * DL],
                        op=ALU.subtract,
                    )
                    dsq = work2.tile([N, DL], F32, tag="dsq")
                    nc.scalar.activation(
                        dsq, dtmp, AF.Square, accum_out=d2t[:, i:i + 1]
                    )
                dcl = work2.tile([N, S], F32, tag="dcl")
                nc.vector.tensor_scalar_max(dcl, d2t, 1e-12)
                dsqr = work2.tile([N, S], F32, tag="dsqr")
                nc.scalar.activation(dsqr, dcl, AF.Sqrt)
                dsg = work2.tile([N, S], F32, tag="dsg")
                nc.scalar.activation(dsg, dsqr, AF.Tanh, scale=0.5)
                # fold [96 (b,j), 8 (i)] -> dD[b + iset*NB, 9*i + j + 1] via PE:
                # for each j: [12, 8(i)] block = esel[:, j-block].T @ dsg
                with tc.tile_pool(name=f"psf{iset}", bufs=1, space="PSUM") as psf_p:
                    psf = psf_p.tile([NB, 64], F32)
                    for j in range(S):
                        nc.tensor.matmul(
                            psf[:, j * S:(j + 1) * S],
                            lhsT=esel_sb[:, j * NB:(j + 1) * NB],
                            rhs=dsg, start=True, stop=True,
                        )
                    dst = dD[iset * 32:iset * 32 + NB, 0:72].rearrange(
                        "p (i j) -> p j i", j=9
                    )[:, 1:9, :]
                    nc.vector.tensor_copy(
                        dst, psf.rearrange("p (j i) -> p j i", i=S)
                    )

        # ---- shortest-path DP on anti-diagonals (9x9 padded grid) ----
        dp = pers.tile([64, 81], F32)
        nc.vector.memset(dp, BIGDP)
        nc.vector.memset(dp[:, 1:2], 0.0)
        for kdiag in range(2, 17):
            lo = max(1, kdiag - 8)
            hi = min(8, kdiag - 1)
            cnt = hi - lo + 1
            f0 = 9 * lo + (kdiag - lo)
            t = pers.tile([64, 8], F32, tag="dptmp", bufs=2)
            nc.vector.tensor_tensor(
                out=t[:, :cnt],
                in0=dp[:, f0 - 9:f0 - 9 + 8 * (cnt - 1) + 1:8],
                in1=dp[:, f0 - 1:f0 - 1 + 8 * (cnt - 1) + 1:8],
                op=ALU.min,
            )
            nc.vector.tensor_tensor(
                out=dp[:, f0:f0 + 8 * (cnt - 1) + 1:8],
                in0=t[:, :cnt],
                in1=dD[:, f0 - 9:f0 - 9 + 8 * (cnt - 1) + 1:8],
                op=ALU.add,
            )

        # ---- loss: relu(ap - an + margin), partial sum over 12 centers ----
        with tc.tile_pool(name="psum3", bufs=1, space="PSUM") as psum3:
            dps = psum3.tile([NB, 1], F32)
            nc.tensor.matmul(dps, lhsT=wsel_sb, rhs=dp[:, 80:81], start=True, stop=True)
            r12 = pers.tile([NB, 1], F32)
            nc.scalar.activation(r12, dps, AF.Relu, bias=marg)
            lsum = psum3.tile([1, 1], F32)
            nc.tensor.matmul(lsum, lhsT=r12, rhs=ones_c, start=True, stop=True)
            out_sb = pers.tile([1, 1], F32)
            nc.vector.tensor_copy(out_sb, lsum)
        nc.sync.dma_start(out, out_sb)


def build_program():
    nc = bacc.Bacc(
        "TRN2", target_bir_lowering=False, debug=False,
        enable_asserts=False, num_devices=NCORES,
    )
    ins = {
        "feats_sl": nc.dram_tensor("feats_sl", [M, DPC], F32, kind="ExternalInput").ap(),
        "lf": nc.dram_tensor("lf", [M, D], F32, kind="ExternalInput").ap(),
        "lab": nc.dram_tensor("lab", [1, M], F32, kind="ExternalInput").ap(),
        "anc": nc.dram_tensor("anc", [NB, 1], F32, kind="ExternalInput").ap(),
        "wsel": nc.dram_tensor("wsel", [64, NB], F32, kind="ExternalInput").ap(),
        "ident": nc.dram_tensor("ident", [128, 128], F32, kind="ExternalInput").ap(),
        "esel": nc.dram_tensor("esel", [N, N], F32, kind="ExternalInput").ap(),
    }
    out = nc.dram_tensor("out", [1, 1], F32, kind="ExternalOutput").ap()
    with tile.TileContext(nc) as tc:
        build_body(tc, out, ins)
    nc.compile()
    return nc


def make_in_maps(feats, labels, local_features):
    feats = np.ascontiguousarray(np.asarray(feats, dtype=np.float32))
    labf = np.asarray(labels).astype(np.float32)
    lf_flat = np.ascontiguousarray(
        np.asarray(local_features, dtype=np.float32).reshape(M, D)
    )
    lab_row = labf.reshape(1, M)
    anchors = labf[::K]  # [96]
    wsel = np.zeros((64, NB), dtype=np.float32)
    wsel[0:NB, :] = np.eye(NB)
    wsel[32:32 + NB, :] = -np.eye(NB)
    ident = np.eye(128, dtype=np.float32)
    esel = np.zeros((N, N), dtype=np.float32)
    for b in range(NB):
        for j in range(S):
            esel[b * S + j, j * NB + b] = 1.0
    in_maps = []
    for c in range(NCORES):
        in_maps.append({
            "feats_sl": np.ascontiguousarray(feats[:, c * DPC:(c + 1) * DPC]),
            "lf": lf_flat,
            "lab": lab_row,
            "anc": np.ascontiguousarray(
                anchors[c * NB:(c + 1) * NB].reshape(NB, 1)
            ),
            "wsel": wsel,
            "ident": ident,
            "esel": esel,
        })
    return in_maps


_NC_CACHE = None


def _get_nc():
    global _NC_CACHE
    if _NC_CACHE is None:
        _NC_CACHE = build_program()
    return _NC_CACHE


def run(feats, labels, local_features, trace=False, **kwargs):
    nc = _get_nc()
    in_maps = make_in_maps(feats, labels, local_features)
    res = bass_utils.run_bass_kernel_spmd(
        nc, in_maps, core_ids=list(range(NCORES)), trace=trace, **kwargs
    )
    partial = sum(float(r["out"][0, 0]) for r in res.results)
    return np.float32(partial / N), res


def kernel(feats, labels, local_features):
    loss, _ = run(feats, labels, local_features)
    return loss


# revision 28
# speedup vs baseline: 5.2791x; 5.2791x over previous
"""Bass/Tile TRN2 kernel for nn_CenterAlignedTripletLoss (8-core SPMD).

Sharding:
  Phase 1 (dist/mining scores): feats sharded along the FEATURE axis
    (8192 -> 1024 per core).  Each core computes a partial mining score
    s[n, m] = ||f_m||^2 - 2 c_n . f_m   over its feature slice
    (row-constant ||c_n||^2 omitted: argmax/argmin per row unaffected),
    accumulated in PSUM via matmuls.  A ReduceScatter(add) over the
    96-center axis gives each core the full scores for its 12 centers.
  Centers: computed per-core in feature-major layout from the same
    feats tiles, transposed to center-major [96, 1024-slice] on PE, and
    exchanged with an AllToAll so core c ends with full-feature centers
    for its 12-center block.
  Phase 2 (mining + local-distance DP): each core mines hardest
    positive/negative for its 12 centers, indirect-DMA gathers the 24
    winning local_features rows, computes the 8x8 stripe distance
    matrices via DVE diff + ACT square-accumulate, applies
    sqrt -> tanh(d/2), runs the shortest-path DP on anti-diagonals, and
    reduces to a per-core partial sum of relu(ap - an + margin).
  Host: sums the 8 partial scalars / 96.
"""

import os
import numpy as np
from contextlib import ExitStack

import concourse.bass as bass
import concourse.bacc as bacc
import concourse.tile as tile
from concourse import mybir
from concourse import bass_utils

F32 = mybir.dt.float32
U32 = mybir.dt.uint32
AF = mybir.ActivationFunctionType
ALU = mybir.AluOpType

NCORES = 8
M = 1536          # samples
D = 8192          # feature dim
DPC = D // NCORES # 1024 features per core
N = 96            # centers
NB = N // NCORES  # 12 centers per core
K = 16            # samples per chunk
S = 8             # stripes
DL = 1024         # local feature dim per stripe
MARGIN = 0.3
BIGM = 1.0e9      # mining mask penalty
BIGDP = 1.0e6     # DP pad value
RG = [list(range(NCORES))]


def build_body(tc, out, ins):
    nc = tc.nc
    feats = ins["feats_sl"]     # [M, DPC] this core's feature slice
    lf = ins["lf"]              # [M, D] full flattened local_features
    lab = ins["lab"]            # [1, M] labels as f32
    anc = ins["anc"]            # [NB, 1] this core's anchor labels (f32)
    wsel = ins["wsel"]          # [64, NB] +I (rows 0-11) / -I (rows 32-43)
    ident = ins["ident"]        # [128, 128] identity
    esel = ins["esel"]          # [96, 96] fold selection
    aavg = ins["aavg"]          # [M, N] chunk-averaging matrix (1/16 entries)

    NT = M // 128               # 12 sample tiles

    with ExitStack() as ctx:
        const = ctx.enter_context(tc.tile_pool(name="const", bufs=1))
        mine = ctx.enter_context(tc.tile_pool(name="mine", bufs=4))
        pers = ctx.enter_context(tc.tile_pool(name="pers", bufs=1))
        dram = ctx.enter_context(tc.tile_pool(name="dram", bufs=1, space="DRAM"))

        # ---- constants ----
        ident_sb = const.tile([128, 128], F32)
        nc.sync.dma_start(ident_sb, ident)
        anc_sb = const.tile([NB, 1], F32)
        nc.sync.dma_start(anc_sb, anc)
        wsel_sb = const.tile([64, NB], F32)
        nc.sync.dma_start(wsel_sb, wsel)
        lab_sb = const.tile([1, M], F32)
        nc.sync.dma_start(lab_sb, lab)
        esel_sb = const.tile([N, N], F32)
        nc.sync.dma_start(esel_sb, esel)
        # averaging matrix tiled [128, 12, 96] (partition p, tile t, center n)
        a_sb = const.tile([128, NT, N], F32)
        nc.sync.dma_start(a_sb, aavg.rearrange("(t p) n -> p t n", p=128))
        ones_n = const.tile([1, N], F32)
        nc.vector.memset(ones_n, 1.0)
        ones_r = const.tile([1, NB], F32)
        nc.vector.memset(ones_r, 1.0)
        ones_c = const.tile([NB, 1], F32)
        nc.vector.memset(ones_c, 1.0)
        marg = const.tile([NB, 1], F32)
        nc.vector.memset(marg, MARGIN)

        ctr_cm = pers.tile([N, DPC], F32)   # center-major centers slice
        # s_sb and x_all are time-disjoint: share one 32KB/partition slot
        s_sb = pers.tile([N, M], F32)   # partial mining scores

        # DRAM bounce buffers for collectives
        rs_in = dram.tile([N, M], F32)
        rs_out = dram.tile([NB, M], F32)
        a2a_in = dram.tile([N, DPC], F32)
        a2a_out = dram.tile([N, DPC], F32)

        # ---- phase 1: natural-layout loads + on-chip transpose ----
        with tc.tile_pool(name="psumc", bufs=1, space="PSUM") as psumc, \
             tc.tile_pool(name="psums", bufs=1, space="PSUM") as psums, \
             tc.tile_pool(name="psumt", bufs=2, space="PSUM") as psumt, \
             tc.tile_pool(name="psumn", bufs=1, space="PSUM") as psumn, \
             tc.tile_pool(name="ftTp", bufs=1) as ftTp, \
             tc.tile_pool(name="kio", bufs=3) as kio, \
             tc.tile_pool(name="workp1", bufs=2) as work:
            ps_c = [psumc.tile([N, 512], F32, name=f"ps_c{h}") for h in range(2)]
            s_ps = [psums.tile([N, 512], F32, name=f"s_ps{b}") for b in range(3)]
            ftT = [ftTp.tile([128, M], F32, name=f"ftT{kd}") for kd in range(8)]
            normcol = pers.tile([128, NT], F32)
            for t in range(NT):
                nt = kio.tile([128, DPC], F32, tag="nt")
                nc.sync.dma_start(nt, feats[t * 128:(t + 1) * 128, :])
                # centers: [96, 1024-slice] center-major via averaging matmul
                for h in range(2):
                    nc.tensor.matmul(
                        ps_c[h], lhsT=a_sb[:, t, :],
                        rhs=nt[:, h * 512:(h + 1) * 512],
                        start=(t == 0), stop=(t == NT - 1),
                    )
                # per-sample squared norms for this tile
                sqd = work.tile([128, DPC], F32, tag="sqd")
                nc.scalar.activation(
                    sqd, nt, AF.Square, accum_out=normcol[:, t:t + 1]
                )
                # transpose the 8 feature blocks into featsT k-tiles
                for kd in range(8):
                    tp = psumt.tile([128, 128], F32, tag="tp")
                    nc.tensor.transpose(
                        tp, nt[:, kd * 128:(kd + 1) * 128], ident_sb
                    )
                    nc.vector.tensor_copy(
                        ftT[kd][:, t * 128:(t + 1) * 128], tp
                    )
            # centers psum -> SBUF (true centers; aavg already has the 1/16)
            for h in range(2):
                nc.vector.tensor_copy(
                    ctr_cm[:, h * 512:(h + 1) * 512], ps_c[h]
                )
            # ctrT blocks scaled by -2 for the dot matmuls
            ctrT2 = []
            for kd in range(8):
                tpc = psumt.tile([128, N], F32, tag="tp")
                nc.tensor.transpose(
                    tpc, ctr_cm[:, kd * 128:(kd + 1) * 128],
                    ident_sb[:N, :N],
                )
                c2 = work.tile([128, N], F32, tag=f"ctrT{kd}", bufs=1)
                nc.vector.tensor_scalar_mul(c2, tpc, -2.0)
                ctrT2.append(c2)
            # norm row [1, 1536] from normcol via transposes
            nrow = pers.tile([1, M], F32)
            for t in range(NT):
                tpn = psumn.tile([1, 128], F32, tag="tpn")
                nc.tensor.transpose(tpn, normcol[:, t:t + 1], ident_sb)
                nc.vector.tensor_copy(nrow[:, t * 128:(t + 1) * 128], tpn)
            # scores: s = norm[m] - 2 c.f accumulated over 8 k-tiles
            for b in range(3):
                for kd in range(8):
                    nc.tensor.matmul(
                        s_ps[b], lhsT=ctrT2[kd],
                        rhs=ftT[kd][:, b * 512:(b + 1) * 512],
                        start=(kd == 0), stop=False,
                    )
                nc.tensor.matmul(
                    s_ps[b], lhsT=ones_n, rhs=nrow[:, b * 512:(b + 1) * 512],
                    start=False, stop=True,
                )
            for b in range(3):
                nc.vector.tensor_copy(s_sb[:, b * 512:(b + 1) * 512], s_ps[b])

        nc.sync.dma_start(rs_in, s_sb)
        nc.sync.dma_start(a2a_in, ctr_cm)
        nc.gpsimd.collective_compute(
            "ReduceScatter", ALU.add, replica_groups=RG,
            ins=[rs_in.opt()], outs=[rs_out.opt()],
        )
        nc.gpsimd.collective_compute(
            "AllToAll", ALU.bypass, replica_groups=RG,
            ins=[a2a_in.opt()], outs=[a2a_out.opt()],
        )

        # ---- X_all prefetch: [96 (b,j-rep), 8192 (i,d)] from a2a_out ----
        # a2a_out row (i*NB + b) holds center b's features [i*1024, (i+1)*1024)
        x_all = pers.tile([N, D], F32)
        x_v = a2a_out.rearrange("(i b) d -> b i d", b=NB)  # [12, 8, 1024]
        for b in range(NB):
            # center b's full row, replicated over the 8 j-partitions
            nc.sync.dma_start(
                x_all[b * S:(b + 1) * S, :],
                x_v[b:b + 1, :, :].to_broadcast([S, S, DPC]),
            )

        # ---- mining on this core's 12 score rows ----
        s12 = mine.tile([NB, M], F32, tag="mine", name="s12")
        nc.sync.dma_start(s12, rs_out)
        diff = mine.tile([NB, M], F32, tag="mine", name="diff")
        m01 = mine.tile([NB, M], F32, tag="mine", name="m01")
        m01c = mine.tile([NB, M], F32, tag="mine", name="m01c")
        pos = mine.tile([NB, M], F32, tag="mine", name="pos")
        neg = mine.tile([NB, M], F32, tag="mine", name="neg")
        with tc.tile_pool(name="psum2", bufs=1, space="PSUM") as psum2:
            lbc = psum2.tile([NB, M], F32)
            for b in range(3):
                nc.tensor.matmul(
                    lbc[:, b * 512:(b + 1) * 512], lhsT=ones_r,
                    rhs=lab_sb[:, b * 512:(b + 1) * 512], start=True, stop=True,
                )
            # diff = L[m] - A[b] per partition
            nc.vector.tensor_scalar(
                out=diff, in0=lbc, scalar1=anc_sb, scalar2=None, op0=ALU.subtract
            )
        nc.vector.tensor_tensor(out=m01, in0=diff, in1=diff, op=ALU.mult)
        nc.vector.tensor_scalar_min(m01c, m01, 1.0)  # 1 iff labels differ
        # pos = s - BIGM*m01c ; neg = BIGM*m01c - BIGM - s
        nc.vector.tensor_scalar(
            out=pos, in0=m01c, scalar1=-BIGM, scalar2=None, op0=ALU.mult
        )
        nc.vector.tensor_tensor(out=pos, in0=pos, in1=s12, op=ALU.add)
        nc.vector.tensor_scalar(
            out=neg, in0=m01c, scalar1=BIGM, scalar2=-BIGM, op0=ALU.mult, op1=ALU.add
        )
        nc.vector.tensor_tensor(out=neg, in0=neg, in1=s12, op=ALU.subtract)

        pmax = pers.tile([NB, 8], F32, tag="pmax")
        pidx = pers.tile([NB, 8], U32)
        nmax = pers.tile([NB, 8], F32, tag="nmax")
        nidx = pers.tile([NB, 8], U32)
        nc.vector.max(pmax, pos)
        nc.vector.max_index(pidx, pmax, pos)
        nc.vector.max(nmax, neg)
        nc.vector.max_index(nidx, nmax, neg)

        # ---- phase 2: gather winners + local distances ----
        # tanh'd dists, padded 9x9 grid; p-set at partitions 0-11, n-set at
        # 32-43 (compute ops need 32-aligned start partitions)
        dD = pers.tile([64, 81], F32)
        nc.vector.memset(dD, 0.0)
        with tc.tile_pool(name="yrawp", bufs=1) as yrawp, \
             tc.tile_pool(name="deip", bufs=1) as deip, \
             tc.tile_pool(name="workp2", bufs=2) as work2:
            for iset, idx in enumerate((pidx, nidx)):
                raw = yrawp.tile([NB, D], F32, tag="yraw", name=f"yraw{iset}")
                nc.gpsimd.indirect_dma_start(
                    out=raw, out_offset=None, in_=lf,
                    in_offset=bass.IndirectOffsetOnAxis(ap=idx[:, :1], axis=0),
                )
                # de-interleave [12, (d, j)] -> [12, (j, d)]
                dei = deip.tile([NB, D], F32, tag="dei", name=f"dei{iset}")
                nc.vector.tensor_copy(
                    out=dei.rearrange("p (j d) -> p j d", j=S),
                    in_=raw.rearrange("p (d j) -> p j d", j=S),
                )
                # fold to 96 partitions: [12, 8192] -> [96 (b,j), 1024 (d)]
                y96 = work2.tile([N, DL], F32, tag="y96")
                nc.sync.dma_start(y96, dei)
                d2t = work2.tile([N, S], F32, tag="d2t")
                for i in range(S):
                    dtmp = work2.tile([N, DL], F32, tag="dtmp")
                    nc.vector.tensor_tensor(
                        out=dtmp, in0=y96, in1=x_all[:, i * DL:(i + 1) * DL],
                        op=ALU.subtract,
                    )
                    dsq = work2.tile([N, DL], F32, tag="dsq")
                    nc.scalar.activation(
                        dsq, dtmp, AF.Square, accum_out=d2t[:, i:i + 1]
                    )
                dcl = work2.tile([N, S], F32, tag="dcl")
                nc.vector.tensor_scalar_max(dcl, d2t, 1e-12)
                dsqr = work2.tile([N, S], F32, tag="dsqr")
                nc.scalar.activation(dsqr, dcl, AF.Sqrt)
                dsg = work2.tile([N, S], F32, tag="dsg")
                nc.scalar.activation(dsg, dsqr, AF.Tanh, scale=0.5)
                # fold [96 (b,j), 8 (i)] -> dD[b + iset*NB, 9*i + j + 1] via PE:
                # for each j: [12, 8(i)] block = esel[:, j-block].T @ dsg
                with tc.tile_pool(name=f"psf{iset}", bufs=1, space="PSUM") as psf_p:
                    psf = psf_p.tile([NB, 64], F32)
                    for j in range(S):
                        nc.tensor.matmul(
                            psf[:, j * S:(j + 1) * S],
                            lhsT=esel_sb[:, j * NB:(j + 1) * NB],
                            rhs=dsg, start=True, stop=True,
                        )
                    dst = dD[iset * 32:iset * 32 + NB, 0:72].rearrange(
                        "p (i j) -> p j i", j=9
                    )[:, 1:9, :]
                    nc.vector.tensor_copy(
                        dst, psf.rearrange("p (j i) -> p j i", i=S)
                    )

        # ---- shortest-path DP on anti-diagonals (9x9 padded grid) ----
        dp = pers.tile([64, 81], F32)
        nc.vector.memset(dp, BIGDP)
        nc.vector.memset(dp[:, 1:2], 0.0)
        for kdiag in range(2, 17):
            lo = max(1, kdiag - 8)
            hi = min(8, kdiag - 1)
            cnt = hi - lo + 1
            f0 = 9 * lo + (kdiag - lo)
            t = pers.tile([64, 8], F32, tag="dptmp", bufs=2)
            nc.vector.tensor_tensor(
                out=t[:, :cnt],
                in0=dp[:, f0 - 9:f0 - 9 + 8 * (cnt - 1) + 1:8],
                in1=dp[:, f0 - 1:f0 - 1 + 8 * (cnt - 1) + 1:8],
                op=ALU.min,
            )
            nc.vector.tensor_tensor(
                out=dp[:, f0:f0 + 8 * (cnt - 1) + 1:8],
                in0=t[:, :cnt],
                in1=dD[:, f0 - 9:f0 - 9 + 8 * (cnt - 1) + 1:8],
                op=ALU.add,
            )

        # ---- loss: relu(ap - an + margin), partial sum over 12 centers ----
        with tc.tile_pool(name="psum3", bufs=1, space="PSUM") as psum3:
            dps = psum3.tile([NB, 1], F32)
            nc.tensor.matmul(dps, lhsT=wsel_sb, rhs=dp[:, 80:81], start=True, stop=True)
            r12 = pers.tile([NB, 1], F32)
            nc.scalar.activation(r12, dps, AF.Relu, bias=marg)
            lsum = psum3.tile([1, 1], F32)
            nc.tensor.matmul(lsum, lhsT=r12, rhs=ones_c, start=True, stop=True)
            out_sb = pers.tile([1, 1], F32)
            nc.vector.tensor_copy(out_sb, lsum)
        nc.sync.dma_start(out, out_sb)


def build_program():
    nc = bacc.Bacc(
        "TRN2", target_bir_lowering=False, debug=False,
        enable_asserts=False, num_devices=NCORES,
    )
    ins = {
        "feats_sl": nc.dram_tensor("feats_sl", [M, DPC], F32, kind="ExternalInput").ap(),
        "lf": nc.dram_tensor("lf", [M, D], F32, kind="ExternalInput").ap(),
        "lab": nc.dram_tensor("lab", [1, M], F32, kind="ExternalInput").ap(),
        "anc": nc.dram_tensor("anc", [NB, 1], F32, kind="ExternalInput").ap(),
        "wsel": nc.dram_tensor("wsel", [64, NB], F32, kind="ExternalInput").ap(),
        "ident": nc.dram_tensor("ident", [128, 128], F32, kind="ExternalInput").ap(),
        "esel": nc.dram_tensor("esel", [N, N], F32, kind="ExternalInput").ap(),
        "aavg": nc.dram_tensor("aavg", [M, N], F32, kind="ExternalInput").ap(),
    }
    out = nc.dram_tensor("out", [1, 1], F32, kind="ExternalOutput").ap()
    with tile.TileContext(nc) as tc:
        build_body(tc, out, ins)
    nc.compile()
    return nc


def make_in_maps(feats, labels, local_features):
    feats = np.ascontiguousarray(np.asarray(feats, dtype=np.float32))
    labf = np.asarray(labels).astype(np.float32)
    lf_flat = np.ascontiguousarray(
        np.asarray(local_features, dtype=np.float32).reshape(M, D)
    )
    lab_row = labf.reshape(1, M)
    anchors = labf[::K]  # [96]
    wsel = np.zeros((64, NB), dtype=np.float32)
    wsel[0:NB, :] = np.eye(NB)
    wsel[32:32 + NB, :] = -np.eye(NB)
    ident = np.eye(128, dtype=np.float32)
    esel = np.zeros((N, N), dtype=np.float32)
    for b in range(NB):
        for j in range(S):
            esel[b * S + j, j * NB + b] = 1.0
    aavg = np.zeros((M, N), dtype=np.float32)
    aavg[np.arange(M), np.arange(M) // K] = 1.0 / K
    in_maps = []
    for c in range(NCORES):
        in_maps.append({
            "feats_sl": np.ascontiguousarray(feats[:, c * DPC:(c + 1) * DPC]),
            "lf": lf_flat,
            "lab": lab_row,
            "anc": np.ascontiguousarray(
                anchors[c * NB:(c + 1) * NB].reshape(NB, 1)
            ),
            "wsel": wsel,
            "ident": ident,
            "esel": esel,
            "aavg": aavg,
        })
    return in_maps


_NC_CACHE = None


def _get_nc():
    global _NC_CACHE
    if _NC_CACHE is None:
        _NC_CACHE = build_program()
    return _NC_CACHE


def run(feats, labels, local_features, trace=False, **kwargs):
    nc = _get_nc()
    in_maps = make_in_maps(feats, labels, local_features)
    res = bass_utils.run_bass_kernel_spmd(
        nc, in_maps, core_ids=list(range(NCORES)), trace=trace, **kwargs
    )
    partial = sum(float(r["out"][0, 0]) for r in res.results)
    return np.float32(partial / N), res


def kernel(feats, labels, local_features):
    loss, _ = run(feats, labels, local_features)
    return loss


# revision 31
# speedup vs baseline: 5.4405x; 1.0306x over previous
"""Bass/Tile TRN2 kernel for nn_CenterAlignedTripletLoss (8-core SPMD).

Sharding:
  Phase 1 (dist/mining scores): feats sharded along the FEATURE axis
    (8192 -> 1024 per core).  Each core computes a partial mining score
    s[n, m] = ||f_m||^2 - 2 c_n . f_m   over its feature slice
    (row-constant ||c_n||^2 omitted: argmax/argmin per row unaffected),
    accumulated in PSUM via matmuls.  A ReduceScatter(add) over the
    96-center axis gives each core the full scores for its 12 centers.
  Centers: computed per-core in feature-major layout from the same
    feats tiles, transposed to center-major [96, 1024-slice] on PE, and
    exchanged with an AllToAll so core c ends with full-feature centers
    for its 12-center block.
  Phase 2 (mining + local-distance DP): each core mines hardest
    positive/negative for its 12 centers, indirect-DMA gathers the 24
    winning local_features rows, computes the 8x8 stripe distance
    matrices via DVE diff + ACT square-accumulate, applies
    sqrt -> tanh(d/2), runs the shortest-path DP on anti-diagonals, and
    reduces to a per-core partial sum of relu(ap - an + margin).
  Host: sums the 8 partial scalars / 96.
"""

import os
import numpy as np
from contextlib import ExitStack

import concourse.bass as bass
import concourse.bacc as bacc
import concourse.tile as tile
from concourse import mybir
from concourse import bass_utils

F32 = mybir.dt.float32
BF16 = mybir.dt.bfloat16
U32 = mybir.dt.uint32
AF = mybir.ActivationFunctionType
ALU = mybir.AluOpType

NCORES = 8
M = 1536          # samples
D = 8192          # feature dim
DPC = D // NCORES # 1024 features per core
N = 96            # centers
NB = N // NCORES  # 12 centers per core
K = 16            # samples per chunk
S = 8             # stripes
DL = 1024         # local feature dim per stripe
MARGIN = 0.3
BIGM = 1.0e9      # mining mask penalty
BIGDP = 1.0e6     # DP pad value
RG = [list(range(NCORES))]


def build_body(tc, out, ins):
    nc = tc.nc
    feats = ins["feats_sl"]     # [M, DPC] this core's feature slice
    lf = ins["lf"]              # [M, D] full flattened local_features
    lab = ins["lab"]            # [1, M] labels as f32
    anc = ins["anc"]            # [NB, 1] this core's anchor labels (f32)
    wsel = ins["wsel"]          # [64, NB] +I (rows 0-11) / -I (rows 32-43)
    ident = ins["ident"]        # [128, 128] identity
    esel = ins["esel"]          # [96, 96] fold selection
    aavg = ins["aavg"]          # [M, N] chunk-averaging matrix (1/16 entries)

    NT = M // 128               # 12 sample tiles

    with ExitStack() as ctx:
        const = ctx.enter_context(tc.tile_pool(name="const", bufs=1))
        pers = ctx.enter_context(tc.tile_pool(name="pers", bufs=1))
        dram = ctx.enter_context(tc.tile_pool(name="dram", bufs=1, space="DRAM"))

        # ---- constants ----
        ident_sb = const.tile([128, 128], F32)
        nc.sync.dma_start(ident_sb, ident)
        anc_sb = const.tile([NB, 1], F32)
        nc.sync.dma_start(anc_sb, anc)
        wsel_sb = const.tile([64, NB], F32)
        nc.sync.dma_start(wsel_sb, wsel)
        lab_sb = const.tile([1, M], F32)
        nc.sync.dma_start(lab_sb, lab)
        esel_sb = const.tile([N, N], F32)
        nc.sync.dma_start(esel_sb, esel)
        # averaging matrix tiled [128, 12, 96] (partition p, tile t, center n)
        a_sb = const.tile([128, NT, N], F32)
        nc.sync.dma_start(a_sb, aavg.rearrange("(t p) n -> p t n", p=128))
        ones_n = const.tile([1, N], F32)
        nc.vector.memset(ones_n, 1.0)
        ident_b = const.tile([128, 128], BF16)
        nc.vector.tensor_copy(ident_b, ident_sb)
        ones_r = const.tile([1, NB], F32)
        nc.vector.memset(ones_r, 1.0)
        ones_c = const.tile([NB, 1], F32)
        nc.vector.memset(ones_c, 1.0)
        marg = const.tile([NB, 1], F32)
        nc.vector.memset(marg, MARGIN)

        ctr_cm = pers.tile([N, DPC], F32)   # center-major centers slice
        # s_sb and x_all are time-disjoint: share one 32KB/partition slot
        s_sb = pers.tile([N, M], F32)   # partial mining scores

        # DRAM bounce buffers for collectives
        rs_in = dram.tile([N, M], F32)
        rs_out = dram.tile([NB, M], F32)
        a2a_in = dram.tile([N, DPC], F32)
        a2a_out = dram.tile([N, DPC], F32)

        # ---- phase 1: natural-layout loads + on-chip transpose ----
        with tc.tile_pool(name="psumc", bufs=1, space="PSUM") as psumc, \
             tc.tile_pool(name="psums", bufs=1, space="PSUM") as psums, \
             tc.tile_pool(name="psumt", bufs=2, space="PSUM") as psumt, \
             tc.tile_pool(name="psumn", bufs=1, space="PSUM") as psumn, \
             tc.tile_pool(name="ftTp", bufs=1) as ftTp, \
             tc.tile_pool(name="kio", bufs=3) as kio, \
             tc.tile_pool(name="workp1", bufs=2) as work:
            ps_c = [psumc.tile([N, 512], F32, name=f"ps_c{h}") for h in range(2)]
            s_ps = [psums.tile([N, 512], F32, name=f"s_ps{b}") for b in range(3)]
            ftT = [ftTp.tile([128, M], BF16, name=f"ftT{kd}") for kd in range(8)]
            normcol = pers.tile([128, NT], F32)
            for t in range(NT):
                nt = kio.tile([128, DPC], F32, tag="nt")
                nc.sync.dma_start(nt, feats[t * 128:(t + 1) * 128, :])
                # centers: [96, 1024-slice] center-major via averaging matmul
                for h in range(2):
                    nc.tensor.matmul(
                        ps_c[h], lhsT=a_sb[:, t, :],
                        rhs=nt[:, h * 512:(h + 1) * 512],
                        start=(t == 0), stop=(t == NT - 1),
                    )
                # per-sample squared norms for this tile
                sqd = work.tile([128, DPC], F32, tag="sqd")
                nc.scalar.activation(
                    sqd, nt, AF.Square, accum_out=normcol[:, t:t + 1]
                )
                # bf16 copy feeds the transposes + dot matmuls (4x PE rate)
                ntb = kio.tile([128, DPC], BF16, tag="ntb")
                nc.scalar.activation(ntb, nt, AF.Copy)
                # transpose the 8 feature blocks into featsT k-tiles
                for kd in range(8):
                    tp = psumt.tile([128, 128], BF16, tag="tp")
                    nc.tensor.transpose(
                        tp, ntb[:, kd * 128:(kd + 1) * 128], ident_b
                    )
                    nc.vector.tensor_copy(
                        ftT[kd][:, t * 128:(t + 1) * 128], tp
                    )
            # centers psum -> SBUF (true centers; aavg already has the 1/16)
            for h in range(2):
                nc.vector.tensor_copy(
                    ctr_cm[:, h * 512:(h + 1) * 512], ps_c[h]
                )
            # ctrT blocks scaled by -2 for the dot matmuls
            ctrT2 = []
            for kd in range(8):
                tpc = psumt.tile([128, N], F32, tag="tp")
                nc.tensor.transpose(
                    tpc, ctr_cm[:, kd * 128:(kd + 1) * 128],
                    ident_sb[:N, :N],
                )
                c2 = work.tile([128, N], BF16, tag=f"ctrT{kd}", bufs=1)
                nc.vector.tensor_scalar_mul(c2, tpc, -2.0)
                ctrT2.append(c2)
            # norm row [1, 1536] from normcol via transposes
            nrow = pers.tile([1, M], F32)
            for t in range(NT):
                tpn = psumn.tile([1, 128], F32, tag="tpn")
                nc.tensor.transpose(tpn, normcol[:, t:t + 1], ident_sb)
                nc.vector.tensor_copy(nrow[:, t * 128:(t + 1) * 128], tpn)
            # scores: s = norm[m] - 2 c.f accumulated over 8 k-tiles
            for kd in range(8):
                for b in range(3):
                    nc.tensor.matmul(
                        s_ps[b], lhsT=ctrT2[kd],
                        rhs=ftT[kd][:, b * 512:(b + 1) * 512],
                        start=(kd == 0), stop=False,
                    )
            for b in range(3):
                nc.tensor.matmul(
                    s_ps[b], lhsT=ones_n, rhs=nrow[:, b * 512:(b + 1) * 512],
                    start=False, stop=True,
                )
            for b in range(3):
                nc.vector.tensor_copy(s_sb[:, b * 512:(b + 1) * 512], s_ps[b])

        nc.sync.dma_start(a2a_in, ctr_cm)
        nc.sync.dma_start(rs_in, s_sb)
        nc.gpsimd.collective_compute(
            "AllToAll", ALU.bypass, replica_groups=RG,
            ins=[a2a_in.opt()], outs=[a2a_out.opt()],
        )
        nc.gpsimd.collective_compute(
            "ReduceScatter", ALU.add, replica_groups=RG,
            ins=[rs_in.opt()], outs=[rs_out.opt()],
        )

        # ---- X_all prefetch: [96 (b,j-rep), 8192 (i,d)] from a2a_out ----
        # a2a_out row (i*NB + b) holds center b's features [i*1024, (i+1)*1024)
        x_all = pers.tile([N, D], F32)
        x_v = a2a_out.rearrange("(i b) d -> b i d", b=NB)  # [12, 8, 1024]
        for b in range(NB):
            # center b's full row, replicated over the 8 j-partitions
            nc.sync.dma_start(
                x_all[b * S:(b + 1) * S, :],
                x_v[b:b + 1, :, :].to_broadcast([S, S, DPC]),
            )

        # ---- mining on this core's 12 score rows ----
        ctx2 = tc.tile_pool(name="mine", bufs=4)
        mine = ctx2.__enter__()
        s12 = mine.tile([NB, M], F32, tag="mine", name="s12")
        nc.sync.dma_start(s12, rs_out)
        diff = mine.tile([NB, M], F32, tag="mine", name="diff")
        m01 = mine.tile([NB, M], F32, tag="mine", name="m01")
        m01c = mine.tile([NB, M], F32, tag="mine", name="m01c")
        pos = mine.tile([NB, M], F32, tag="mine", name="pos")
        neg = mine.tile([NB, M], F32, tag="mine", name="neg")
        with tc.tile_pool(name="psum2", bufs=1, space="PSUM") as psum2:
            lbc = psum2.tile([NB, M], F32)
            for b in range(3):
                nc.tensor.matmul(
                    lbc[:, b * 512:(b + 1) * 512], lhsT=ones_r,
                    rhs=lab_sb[:, b * 512:(b + 1) * 512], start=True, stop=True,
                )
            # diff = L[m] - A[b] per partition
            nc.vector.tensor_scalar(
                out=diff, in0=lbc, scalar1=anc_sb, scalar2=None, op0=ALU.subtract
            )
        nc.vector.tensor_tensor(out=m01, in0=diff, in1=diff, op=ALU.mult)
        nc.vector.tensor_scalar_min(m01c, m01, 1.0)  # 1 iff labels differ
        # pos = s - BIGM*m01c ; neg = BIGM*m01c - BIGM - s
        nc.vector.tensor_scalar(
            out=pos, in0=m01c, scalar1=-BIGM, scalar2=None, op0=ALU.mult
        )
        nc.vector.tensor_tensor(out=pos, in0=pos, in1=s12, op=ALU.add)
        nc.vector.tensor_scalar(
            out=neg, in0=m01c, scalar1=BIGM, scalar2=-BIGM, op0=ALU.mult, op1=ALU.add
        )
        nc.vector.tensor_tensor(out=neg, in0=neg, in1=s12, op=ALU.subtract)

        pmax = pers.tile([NB, 8], F32, tag="pmax")
        pidx = pers.tile([NB, 8], U32)
        nmax = pers.tile([NB, 8], F32, tag="nmax")
        nidx = pers.tile([NB, 8], U32)
        nc.vector.max(pmax, pos)
        nc.vector.max_index(pidx, pmax, pos)
        nc.vector.max(nmax, neg)
        nc.vector.max_index(nidx, nmax, neg)
        ctx2.__exit__(None, None, None)

        # ---- phase 2: gather winners + local distances ----
        # tanh'd dists, padded 9x9 grid; p-set at partitions 0-11, n-set at
        # 32-43 (compute ops need 32-aligned start partitions)
        dD = pers.tile([64, 81], F32)
        nc.vector.memset(dD, 0.0)
        with tc.tile_pool(name="yrawp", bufs=2) as yrawp, \
             tc.tile_pool(name="deip", bufs=1) as deip, \
             tc.tile_pool(name="workp2", bufs=2) as work2:
            for iset, idx in enumerate((pidx, nidx)):
                raw = yrawp.tile([NB, D], F32, tag="yraw", name=f"yraw{iset}")
                nc.gpsimd.indirect_dma_start(
                    out=raw, out_offset=None, in_=lf,
                    in_offset=bass.IndirectOffsetOnAxis(ap=idx[:, :1], axis=0),
                )
                # de-interleave [12, (d, j)] -> [12, (j, d)]; split ACT/DVE
                dei = deip.tile([NB, D], F32, tag="dei", name=f"dei{iset}")
                dei_v = dei.rearrange("p (j d) -> p j d", j=S)
                raw_v = raw.rearrange("p (d j) -> p j d", j=S)
                nc.scalar.activation(dei_v[:, 0:4, :], raw_v[:, 0:4, :], AF.Copy)
                nc.vector.tensor_copy(dei_v[:, 4:8, :], raw_v[:, 4:8, :])
                # fold to 96 partitions: [12, 8192] -> [96 (b,j), 1024 (d)]
                y96 = work2.tile([N, DL], F32, tag="y96")
                nc.sync.dma_start(y96, dei)
                d2t = work2.tile([N, S], F32, tag="d2t")
                for i in range(S):
                    dtmp = work2.tile([N, DL], F32, tag="dtmp")
                    nc.vector.tensor_tensor(
                        out=dtmp, in0=y96, in1=x_all[:, i * DL:(i + 1) * DL],
                        op=ALU.subtract,
                    )
                    dsq = work2.tile([N, DL], F32, tag="dsq")
                    nc.scalar.activation(
                        dsq, dtmp, AF.Square, accum_out=d2t[:, i:i + 1]
                    )
                dcl = work2.tile([N, S], F32, tag="dcl")
                nc.vector.tensor_scalar_max(dcl, d2t, 1e-12)
                dsqr = work2.tile([N, S], F32, tag="dsqr")
                nc.scalar.activation(dsqr, dcl, AF.Sqrt)
                dsg = work2.tile([N, S], F32, tag="dsg")
                nc.scalar.activation(dsg, dsqr, AF.Tanh, scale=0.5)
                # fold [96 (b,j), 8 (i)] -> dD[b + iset*NB, 9*i + j + 1] via PE:
                # for each j: [12, 8(i)] block = esel[:, j-block].T @ dsg
                with tc.tile_pool(name=f"psf{iset}", bufs=1, space="PSUM") as psf_p:
                    psf = psf_p.tile([NB, 64], F32)
                    for j in range(S):
                        nc.tensor.matmul(
                            psf[:, j * S:(j + 1) * S],
                            lhsT=esel_sb[:, j * NB:(j + 1) * NB],
                            rhs=dsg, start=True, stop=True,
                        )
                    dst = dD[iset * 32:iset * 32 + NB, 0:72].rearrange(
                        "p (i j) -> p j i", j=9
                    )[:, 1:9, :]
                    nc.vector.tensor_copy(
                        dst, psf.rearrange("p (j i) -> p j i", i=S)
                    )

        # ---- shortest-path DP on anti-diagonals (9x9 padded grid) ----
        dp = pers.tile([64, 81], F32)
        nc.vector.memset(dp, BIGDP)
        nc.vector.memset(dp[:, 1:2], 0.0)
        for kdiag in range(2, 17):
            lo = max(1, kdiag - 8)
            hi = min(8, kdiag - 1)
            cnt = hi - lo + 1
            f0 = 9 * lo + (kdiag - lo)
            t = pers.tile([64, 8], F32, tag="dptmp", bufs=2)
            nc.vector.tensor_tensor(
                out=t[:, :cnt],
                in0=dp[:, f0 - 9:f0 - 9 + 8 * (cnt - 1) + 1:8],
                in1=dp[:, f0 - 1:f0 - 1 + 8 * (cnt - 1) + 1:8],
                op=ALU.min,
            )
            nc.vector.tensor_tensor(
                out=dp[:, f0:f0 + 8 * (cnt - 1) + 1:8],
                in0=t[:, :cnt],
                in1=dD[:, f0 - 9:f0 - 9 + 8 * (cnt - 1) + 1:8],
                op=ALU.add,
            )

        # ---- loss: relu(ap - an + margin), partial sum over 12 centers ----
        with tc.tile_pool(name="psum3", bufs=1, space="PSUM") as psum3:
            dps = psum3.tile([NB, 1], F32)
            nc.tensor.matmul(dps, lhsT=wsel_sb, rhs=dp[:, 80:81], start=True, stop=True)
            r12 = pers.tile([NB, 1], F32)
            nc.scalar.activation(r12, dps, AF.Relu, bias=marg)
            lsum = psum3.tile([1, 1], F32)
            nc.tensor.matmul(lsum, lhsT=r12, rhs=ones_c, start=True, stop=True)
            out_sb = pers.tile([1, 1], F32)
            nc.vector.tensor_copy(out_sb, lsum)
        nc.sync.dma_start(out, out_sb)


def build_program():
    nc = bacc.Bacc(
        "TRN2", target_bir_lowering=False, debug=False,
        enable_asserts=False, num_devices=NCORES,
    )
    ins = {
        "feats_sl": nc.dram_tensor("feats_sl", [M, DPC], F32, kind="ExternalInput").ap(),
        "lf": nc.dram_tensor("lf", [M, D], F32, kind="ExternalInput").ap(),
        "lab": nc.dram_tensor("lab", [1, M], F32, kind="ExternalInput").ap(),
        "anc": nc.dram_tensor("anc", [NB, 1], F32, kind="ExternalInput").ap(),
        "wsel": nc.dram_tensor("wsel", [64, NB], F32, kind="ExternalInput").ap(),
        "ident": nc.dram_tensor("ident", [128, 128], F32, kind="ExternalInput").ap(),
        "esel": nc.dram_tensor("esel", [N, N], F32, kind="ExternalInput").ap(),
        "aavg": nc.dram_tensor("aavg", [M, N], F32, kind="ExternalInput").ap(),
    }
    out = nc.dram_tensor("out", [1, 1], F32, kind="ExternalOutput").ap()
    with tile.TileContext(nc) as tc:
        build_body(tc, out, ins)
    nc.compile()
    return nc


def make_in_maps(feats, labels, local_features):
    feats = np.ascontiguousarray(np.asarray(feats, dtype=np.float32))
    labf = np.asarray(labels).astype(np.float32)
    lf_flat = np.ascontiguousarray(
        np.asarray(local_features, dtype=np.float32).reshape(M, D)
    )
    lab_row = labf.reshape(1, M)
    anchors = labf[::K]  # [96]
    wsel = np.zeros((64, NB), dtype=np.float32)
    wsel[0:NB, :] = np.eye(NB)
    wsel[32:32 + NB, :] = -np.eye(NB)
    ident = np.eye(128, dtype=np.float32)
    esel = np.zeros((N, N), dtype=np.float32)
    for b in range(NB):
        for j in range(S):
            esel[b * S + j, j * NB + b] = 1.0
    aavg = np.zeros((M, N), dtype=np.float32)
    aavg[np.arange(M), np.arange(M) // K] = 1.0 / K
    in_maps = []
    for c in range(NCORES):
        in_maps.append({
            "feats_sl": np.ascontiguousarray(feats[:, c * DPC:(c + 1) * DPC]),
            "lf": lf_flat,
            "lab": lab_row,
            "anc": np.ascontiguousarray(
                anchors[c * NB:(c + 1) * NB].reshape(NB, 1)
            ),
            "wsel": wsel,
            "ident": ident,
            "esel": esel,
            "aavg": aavg,
        })
    return in_maps


_NC_CACHE = None


def _get_nc():
    global _NC_CACHE
    if _NC_CACHE is None:
        _NC_CACHE = build_program()
    return _NC_CACHE


def run(feats, labels, local_features, trace=False, **kwargs):
    nc = _get_nc()
    in_maps = make_in_maps(feats, labels, local_features)
    res = bass_utils.run_bass_kernel_spmd(
        nc, in_maps, core_ids=list(range(NCORES)), trace=trace, **kwargs
    )
    partial = sum(float(r["out"][0, 0]) for r in res.results)
    return np.float32(partial / N), res


def kernel(feats, labels, local_features):
    loss, _ = run(feats, labels, local_features)
    return loss


# revision 33
# speedup vs baseline: 5.6779x; 1.0436x over previous
"""Bass/Tile TRN2 kernel for nn_CenterAlignedTripletLoss (8-core SPMD).

Sharding:
  Phase 1 (dist/mining scores): feats sharded along the FEATURE axis
    (8192 -> 1024 per core).  Each core computes a partial mining score
    s[n, m] = ||f_m||^2 - 2 c_n . f_m   over its feature slice
    (row-constant ||c_n||^2 omitted: argmax/argmin per row unaffected),
    accumulated in PSUM via matmuls.  A ReduceScatter(add) over the
    96-center axis gives each core the full scores for its 12 centers.
  Centers: computed per-core in feature-major layout from the same
    feats tiles, transposed to center-major [96, 1024-slice] on PE, and
    exchanged with an AllToAll so core c ends with full-feature centers
    for its 12-center block.
  Phase 2 (mining + local-distance DP): each core mines hardest
    positive/negative for its 12 centers, indirect-DMA gathers the 24
    winning local_features rows, computes the 8x8 stripe distance
    matrices via DVE diff + ACT square-accumulate, applies
    sqrt -> tanh(d/2), runs the shortest-path DP on anti-diagonals, and
    reduces to a per-core partial sum of relu(ap - an + margin).
  Host: sums the 8 partial scalars / 96.
"""

import os
import numpy as np
from contextlib import ExitStack

import concourse.bass as bass
import concourse.bacc as bacc
import concourse.tile as tile
from concourse import mybir
from concourse import bass_utils

F32 = mybir.dt.float32
BF16 = mybir.dt.bfloat16
U32 = mybir.dt.uint32
AF = mybir.ActivationFunctionType
ALU = mybir.AluOpType

NCORES = 8
M = 1536          # samples
D = 8192          # feature dim
DPC = D // NCORES # 1024 features per core
N = 96            # centers
NB = N // NCORES  # 12 centers per core
K = 16            # samples per chunk
S = 8             # stripes
DL = 1024         # local feature dim per stripe
MARGIN = 0.3
BIGM = 1.0e9      # mining mask penalty
BIGDP = 1.0e6     # DP pad value
RG = [list(range(NCORES))]


def build_body(tc, out, ins):
    nc = tc.nc
    feats = ins["feats_sl"]     # [M, DPC] this core's feature slice (bf16)
    lf = ins["lf"]              # [M, D] full flattened local_features
    lab = ins["lab"]            # [1, M] labels as f32
    anc = ins["anc"]            # [NB, 1] this core's anchor labels (f32)
    wsel = ins["wsel"]          # [64, NB] +I (rows 0-11) / -I (rows 32-43)
    ident = ins["ident"]        # [128, 128] identity
    esel = ins["esel"]          # [96, 96] fold selection
    aavg = ins["aavg"]          # [M, N] chunk-averaging matrix (1/16 entries)

    NT = M // 128               # 12 sample tiles

    with ExitStack() as ctx:
        const = ctx.enter_context(tc.tile_pool(name="const", bufs=1))
        pers = ctx.enter_context(tc.tile_pool(name="pers", bufs=1))
        dram = ctx.enter_context(tc.tile_pool(name="dram", bufs=1, space="DRAM"))

        # ---- constants ----
        ident_sb = const.tile([128, 128], F32)
        nc.sync.dma_start(ident_sb, ident)
        anc_sb = const.tile([NB, 1], F32)
        nc.sync.dma_start(anc_sb, anc)
        wsel_sb = const.tile([64, NB], F32)
        nc.sync.dma_start(wsel_sb, wsel)
        lab_sb = const.tile([1, M], F32)
        nc.sync.dma_start(lab_sb, lab)
        esel_sb = const.tile([N, N], F32)
        nc.sync.dma_start(esel_sb, esel)
        # averaging matrix tiled [128, 12, 96] (partition p, tile t, center n)
        a_sb = const.tile([128, NT, N], BF16)
        nc.sync.dma_start(a_sb, aavg.rearrange("(t p) n -> p t n", p=128))
        ones_n = const.tile([1, N], F32)
        nc.vector.memset(ones_n, 1.0)
        ones_r = const.tile([1, NB], F32)
        nc.vector.memset(ones_r, 1.0)
        ones_c = const.tile([NB, 1], F32)
        nc.vector.memset(ones_c, 1.0)
        marg = const.tile([NB, 1], F32)
        nc.vector.memset(marg, MARGIN)

        ctr_cm = pers.tile([N, DPC], F32)   # center-major centers slice
        # s_sb and x_all are time-disjoint: share one 32KB/partition slot
        s_sb = pers.tile([N, M], F32)   # partial mining scores

        # DRAM bounce buffers for collectives
        rs_in = dram.tile([N, M], F32)
        rs_out = dram.tile([NB, M], F32)
        a2a_in = dram.tile([N, DPC], F32)
        a2a_out = dram.tile([N, DPC], F32)

        # ---- phase 1: natural-layout loads + on-chip transpose ----
        with tc.tile_pool(name="psumc", bufs=1, space="PSUM") as psumc, \
             tc.tile_pool(name="psums", bufs=1, space="PSUM") as psums, \
             tc.tile_pool(name="psumt", bufs=2, space="PSUM") as psumt, \
             tc.tile_pool(name="psumn", bufs=1, space="PSUM") as psumn, \
             tc.tile_pool(name="ftTp", bufs=1) as ftTp, \
             tc.tile_pool(name="kio", bufs=3) as kio, \
             tc.tile_pool(name="workp1", bufs=2) as work:
            ps_c = [psumc.tile([N, 512], F32, name=f"ps_c{h}") for h in range(2)]
            s_ps = [psums.tile([N, 512], F32, name=f"s_ps{b}") for b in range(3)]
            ftT = [ftTp.tile([128, M], BF16, name=f"ftT{kd}") for kd in range(8)]
            normcol = pers.tile([128, NT], F32)
            # featsT k-tiles via hardware DMA transpose (bf16, xbar)
            for kd in range(8):
                nc.sync.dma_start_transpose(
                    ftT[kd], feats[:, kd * 128:(kd + 1) * 128]
                )
            for t in range(NT):
                nt = kio.tile([128, DPC], BF16, tag="nt")
                nc.sync.dma_start(nt, feats[t * 128:(t + 1) * 128, :])
                # centers: [96, 1024-slice] center-major via averaging matmul
                for h in range(2):
                    nc.tensor.matmul(
                        ps_c[h], lhsT=a_sb[:, t, :],
                        rhs=nt[:, h * 512:(h + 1) * 512],
                        start=(t == 0), stop=(t == NT - 1),
                    )
                # per-sample squared norms for this tile
                sqd = work.tile([128, DPC], F32, tag="sqd")
                nc.scalar.activation(
                    sqd, nt, AF.Square, accum_out=normcol[:, t:t + 1]
                )
            # centers psum -> SBUF (true centers; aavg already has the 1/16)
            for h in range(2):
                nc.vector.tensor_copy(
                    ctr_cm[:, h * 512:(h + 1) * 512], ps_c[h]
                )
            # ctrT blocks scaled by -2 for the dot matmuls
            ctrT2 = []
            for kd in range(8):
                tpc = psumt.tile([128, N], F32, tag="tp")
                nc.tensor.transpose(
                    tpc, ctr_cm[:, kd * 128:(kd + 1) * 128],
                    ident_sb[:N, :N],
                )
                c2 = work.tile([128, N], BF16, tag=f"ctrT{kd}", bufs=1)
                nc.vector.tensor_scalar_mul(c2, tpc, -2.0)
                ctrT2.append(c2)
            # norm row [1, 1536] from normcol via transposes
            nrow = pers.tile([1, M], F32)
            for t in range(NT):
                tpn = psumn.tile([1, 128], F32, tag="tpn")
                nc.tensor.transpose(tpn, normcol[:, t:t + 1], ident_sb)
                nc.vector.tensor_copy(nrow[:, t * 128:(t + 1) * 128], tpn)
            # scores: s = norm[m] - 2 c.f accumulated over 8 k-tiles
            for kd in range(8):
                for b in range(3):
                    nc.tensor.matmul(
                        s_ps[b], lhsT=ctrT2[kd],
                        rhs=ftT[kd][:, b * 512:(b + 1) * 512],
                        start=(kd == 0), stop=False,
                    )
            for b in range(3):
                nc.tensor.matmul(
                    s_ps[b], lhsT=ones_n, rhs=nrow[:, b * 512:(b + 1) * 512],
                    start=False, stop=True,
                )
            for b in range(3):
                nc.vector.tensor_copy(s_sb[:, b * 512:(b + 1) * 512], s_ps[b])

        nc.sync.dma_start(a2a_in, ctr_cm)
        nc.sync.dma_start(rs_in, s_sb)
        nc.gpsimd.collective_compute(
            "AllToAll", ALU.bypass, replica_groups=RG,
            ins=[a2a_in.opt()], outs=[a2a_out.opt()],
        )
        nc.gpsimd.collective_compute(
            "ReduceScatter", ALU.add, replica_groups=RG,
            ins=[rs_in.opt()], outs=[rs_out.opt()],
        )

        # ---- X_all prefetch: [96 (b,j-rep), 8192 (i,d)] from a2a_out ----
        # a2a_out row (i*NB + b) holds center b's features [i*1024, (i+1)*1024)
        x_all = pers.tile([N, D], F32)
        x_v = a2a_out.rearrange("(i b) d -> b i d", b=NB)  # [12, 8, 1024]
        for b in range(NB):
            # center b's full row, replicated over the 8 j-partitions
            nc.sync.dma_start(
                x_all[b * S:(b + 1) * S, :],
                x_v[b:b + 1, :, :].to_broadcast([S, S, DPC]),
            )

        # ---- mining on this core's 12 score rows ----
        ctx2 = tc.tile_pool(name="mine", bufs=4)
        mine = ctx2.__enter__()
        s12 = mine.tile([NB, M], F32, tag="mine", name="s12")
        nc.sync.dma_start(s12, rs_out)
        diff = mine.tile([NB, M], F32, tag="mine", name="diff")
        m01 = mine.tile([NB, M], F32, tag="mine", name="m01")
        m01c = mine.tile([NB, M], F32, tag="mine", name="m01c")
        pos = mine.tile([NB, M], F32, tag="mine", name="pos")
        neg = mine.tile([NB, M], F32, tag="mine", name="neg")
        with tc.tile_pool(name="psum2", bufs=1, space="PSUM") as psum2:
            lbc = psum2.tile([NB, M], F32)
            for b in range(3):
                nc.tensor.matmul(
                    lbc[:, b * 512:(b + 1) * 512], lhsT=ones_r,
                    rhs=lab_sb[:, b * 512:(b + 1) * 512], start=True, stop=True,
                )
            # diff = L[m] - A[b] per partition
            nc.vector.tensor_scalar(
                out=diff, in0=lbc, scalar1=anc_sb, scalar2=None, op0=ALU.subtract
            )
        nc.vector.tensor_tensor(out=m01, in0=diff, in1=diff, op=ALU.mult)
        nc.vector.tensor_scalar_min(m01c, m01, 1.0)  # 1 iff labels differ
        # pos = s - BIGM*m01c ; neg = BIGM*m01c - BIGM - s
        nc.vector.tensor_scalar(
            out=pos, in0=m01c, scalar1=-BIGM, scalar2=None, op0=ALU.mult
        )
        nc.vector.tensor_tensor(out=pos, in0=pos, in1=s12, op=ALU.add)
        nc.vector.tensor_scalar(
            out=neg, in0=m01c, scalar1=BIGM, scalar2=-BIGM, op0=ALU.mult, op1=ALU.add
        )
        nc.vector.tensor_tensor(out=neg, in0=neg, in1=s12, op=ALU.subtract)

        pmax = pers.tile([NB, 8], F32, tag="pmax")
        pidx = pers.tile([NB, 8], U32)
        nmax = pers.tile([NB, 8], F32, tag="nmax")
        nidx = pers.tile([NB, 8], U32)
        nc.vector.max(pmax, pos)
        nc.vector.max_index(pidx, pmax, pos)
        nc.vector.max(nmax, neg)
        nc.vector.max_index(nidx, nmax, neg)
        ctx2.__exit__(None, None, None)

        # ---- phase 2: gather winners + local distances ----
        # tanh'd dists, padded 9x9 grid; p-set at partitions 0-11, n-set at
        # 32-43 (compute ops need 32-aligned start partitions)
        dD = pers.tile([64, 81], F32)
        nc.vector.memset(dD, 0.0)
        with tc.tile_pool(name="yrawp", bufs=2) as yrawp, \
             tc.tile_pool(name="deip", bufs=1) as deip, \
             tc.tile_pool(name="workp2", bufs=2) as work2:
            for iset, idx in enumerate((pidx, nidx)):
                raw = yrawp.tile([NB, D], F32, tag="yraw", name=f"yraw{iset}")
                nc.gpsimd.indirect_dma_start(
                    out=raw, out_offset=None, in_=lf,
                    in_offset=bass.IndirectOffsetOnAxis(ap=idx[:, :1], axis=0),
                )
                # de-interleave [12, (d, j)] -> [12, (j, d)]; split ACT/DVE
                dei = deip.tile([NB, D], F32, tag="dei", name=f"dei{iset}")
                dei_v = dei.rearrange("p (j d) -> p j d", j=S)
                raw_v = raw.rearrange("p (d j) -> p j d", j=S)
                nc.scalar.activation(dei_v[:, 0:4, :], raw_v[:, 0:4, :], AF.Copy)
                nc.vector.tensor_copy(dei_v[:, 4:8, :], raw_v[:, 4:8, :])
                # fold to 96 partitions: [12, 8192] -> [96 (b,j), 1024 (d)]
                y96 = work2.tile([N, DL], F32, tag="y96")
                nc.sync.dma_start(y96, dei)
                d2t = work2.tile([N, S], F32, tag="d2t")
                for i in range(S):
                    dtmp = work2.tile([N, DL], F32, tag="dtmp")
                    nc.vector.tensor_tensor(
                        out=dtmp, in0=y96, in1=x_all[:, i * DL:(i + 1) * DL],
                        op=ALU.subtract,
                    )
                    dsq = work2.tile([N, DL], F32, tag="dsq")
                    nc.scalar.activation(
                        dsq, dtmp, AF.Square, accum_out=d2t[:, i:i + 1]
                    )
                dcl = work2.tile([N, S], F32, tag="dcl")
                nc.vector.tensor_scalar_max(dcl, d2t, 1e-12)
                dsqr = work2.tile([N, S], F32, tag="dsqr")
                nc.scalar.activation(dsqr, dcl, AF.Sqrt)
                dsg = work2.tile([N, S], F32, tag="dsg")
                nc.scalar.activation(dsg, dsqr, AF.Tanh, scale=0.5)
                # fold [96 (b,j), 8 (i)] -> dD[b + iset*NB, 9*i + j + 1] via PE:
                # for each j: [12, 8(i)] block = esel[:, j-block].T @ dsg
                with tc.tile_pool(name=f"psf{iset}", bufs=1, space="PSUM") as psf_p:
                    psf = psf_p.tile([NB, 64], F32)
                    for j in range(S):
                        nc.tensor.matmul(
                            psf[:, j * S:(j + 1) * S],
                            lhsT=esel_sb[:, j * NB:(j + 1) * NB],
                            rhs=dsg, start=True, stop=True,
                        )
                    dst = dD[iset * 32:iset * 32 + NB, 0:72].rearrange(
                        "p (i j) -> p j i", j=9
                    )[:, 1:9, :]
                    nc.vector.tensor_copy(
                        dst, psf.rearrange("p (j i) -> p j i", i=S)
                    )

        # ---- shortest-path DP on anti-diagonals (9x9 padded grid) ----
        dp = pers.tile([64, 81], F32)
        nc.vector.memset(dp, BIGDP)
        nc.vector.memset(dp[:, 1:2], 0.0)
        for kdiag in range(2, 17):
            lo = max(1, kdiag - 8)
            hi = min(8, kdiag - 1)
            cnt = hi - lo + 1
            f0 = 9 * lo + (kdiag - lo)
            t = pers.tile([64, 8], F32, tag="dptmp", bufs=2)
            nc.vector.tensor_tensor(
                out=t[:, :cnt],
                in0=dp[:, f0 - 9:f0 - 9 + 8 * (cnt - 1) + 1:8],
                in1=dp[:, f0 - 1:f0 - 1 + 8 * (cnt - 1) + 1:8],
                op=ALU.min,
            )
            nc.vector.tensor_tensor(
                out=dp[:, f0:f0 + 8 * (cnt - 1) + 1:8],
                in0=t[:, :cnt],
                in1=dD[:, f0 - 9:f0 - 9 + 8 * (cnt - 1) + 1:8],
                op=ALU.add,
            )

        # ---- loss: relu(ap - an + margin), partial sum over 12 centers ----
        with tc.tile_pool(name="psum3", bufs=1, space="PSUM") as psum3:
            dps = psum3.tile([NB, 1], F32)
            nc.tensor.matmul(dps, lhsT=wsel_sb, rhs=dp[:, 80:81], start=True, stop=True)
            r12 = pers.tile([NB, 1], F32)
            nc.scalar.activation(r12, dps, AF.Relu, bias=marg)
            lsum = psum3.tile([1, 1], F32)
            nc.tensor.matmul(lsum, lhsT=r12, rhs=ones_c, start=True, stop=True)
            out_sb = pers.tile([1, 1], F32)
            nc.vector.tensor_copy(out_sb, lsum)
        nc.sync.dma_start(out, out_sb)


def build_program():
    nc = bacc.Bacc(
        "TRN2", target_bir_lowering=False, debug=False,
        enable_asserts=False, num_devices=NCORES,
    )
    ins = {
        "feats_sl": nc.dram_tensor("feats_sl", [M, DPC], BF16, kind="ExternalInput").ap(),
        "lf": nc.dram_tensor("lf", [M, D], F32, kind="ExternalInput").ap(),
        "lab": nc.dram_tensor("lab", [1, M], F32, kind="ExternalInput").ap(),
        "anc": nc.dram_tensor("anc", [NB, 1], F32, kind="ExternalInput").ap(),
        "wsel": nc.dram_tensor("wsel", [64, NB], F32, kind="ExternalInput").ap(),
        "ident": nc.dram_tensor("ident", [128, 128], F32, kind="ExternalInput").ap(),
        "esel": nc.dram_tensor("esel", [N, N], F32, kind="ExternalInput").ap(),
        "aavg": nc.dram_tensor("aavg", [M, N], BF16, kind="ExternalInput").ap(),
    }
    out = nc.dram_tensor("out", [1, 1], F32, kind="ExternalOutput").ap()
    with tile.TileContext(nc) as tc:
        build_body(tc, out, ins)
    nc.compile()
    return nc


def make_in_maps(feats, labels, local_features):
    bf16 = mybir.dt.np(BF16)
    feats = np.asarray(feats, dtype=np.float32).astype(bf16)
    labf = np.asarray(labels).astype(np.float32)
    lf_flat = np.ascontiguousarray(
        np.asarray(local_features, dtype=np.float32).reshape(M, D)
    )
    lab_row = labf.reshape(1, M)
    anchors = labf[::K]  # [96]
    wsel = np.zeros((64, NB), dtype=np.float32)
    wsel[0:NB, :] = np.eye(NB)
    wsel[32:32 + NB, :] = -np.eye(NB)
    ident = np.eye(128, dtype=np.float32)
    esel = np.zeros((N, N), dtype=np.float32)
    for b in range(NB):
        for j in range(S):
            esel[b * S + j, j * NB + b] = 1.0
    aavg = np.zeros((M, N), dtype=np.float32)
    aavg[np.arange(M), np.arange(M) // K] = 1.0 / K
    aavg = aavg.astype(bf16)
    in_maps = []
    for c in range(NCORES):
        in_maps.append({
            "feats_sl": np.ascontiguousarray(feats[:, c * DPC:(c + 1) * DPC]),
            "lf": lf_flat,
            "lab": lab_row,
            "anc": np.ascontiguousarray(
                anchors[c * NB:(c + 1) * NB].reshape(NB, 1)
            ),
            "wsel": wsel,
            "ident": ident,
            "esel": esel,
            "aavg": aavg,
        })
    return in_maps


_NC_CACHE = None


def _get_nc():
    global _NC_CACHE
    if _NC_CACHE is None:
        _NC_CACHE = build_program()
    return _NC_CACHE


def run(feats, labels, local_features, trace=False, **kwargs):
    nc = _get_nc()
    in_maps = make_in_maps(feats, labels, local_features)
    res = bass_utils.run_bass_kernel_spmd(
        nc, in_maps, core_ids=list(range(NCORES)), trace=trace, **kwargs
    )
    partial = sum(float(r["out"][0, 0]) for r in res.results)
    return np.float32(partial / N), res


def kernel(feats, labels, local_features):
    loss, _ = run(feats, labels, local_features)
    return loss


# revision 34
# speedup vs baseline: 6.0468x; 1.0650x over previous
"""Bass/Tile TRN2 kernel for nn_CenterAlignedTripletLoss (8-core SPMD).

Sharding:
  Phase 1 (dist/mining scores): feats sharded along the FEATURE axis
    (8192 -> 1024 per core).  Each core computes a partial mining score
    s[n, m] = ||f_m||^2 - 2 c_n . f_m   over its feature slice
    (row-constant ||c_n||^2 omitted: argmax/argmin per row unaffected),
    accumulated in PSUM via matmuls.  A ReduceScatter(add) over the
    96-center axis gives each core the full scores for its 12 centers.
  Centers: computed per-core in feature-major layout from the same
    feats tiles, transposed to center-major [96, 1024-slice] on PE, and
    exchanged with an AllToAll so core c ends with full-feature centers
    for its 12-center block.
  Phase 2 (mining + local-distance DP): each core mines hardest
    positive/negative for its 12 centers, indirect-DMA gathers the 24
    winning local_features rows, computes the 8x8 stripe distance
    matrices via DVE diff + ACT square-accumulate, applies
    sqrt -> tanh(d/2), runs the shortest-path DP on anti-diagonals, and
    reduces to a per-core partial sum of relu(ap - an + margin).
  Host: sums the 8 partial scalars / 96.
"""

import os
import numpy as np
from contextlib import ExitStack

import concourse.bass as bass
import concourse.bacc as bacc
import concourse.tile as tile
from concourse import mybir
from concourse import bass_utils

F32 = mybir.dt.float32
BF16 = mybir.dt.bfloat16
U32 = mybir.dt.uint32
AF = mybir.ActivationFunctionType
ALU = mybir.AluOpType

NCORES = 8
M = 1536          # samples
D = 8192          # feature dim
DPC = D // NCORES # 1024 features per core
N = 96            # centers
NB = N // NCORES  # 12 centers per core
K = 16            # samples per chunk
S = 8             # stripes
DL = 1024         # local feature dim per stripe
MARGIN = 0.3
BIGM = 1.0e9      # mining mask penalty
BIGDP = 1.0e6     # DP pad value
RG = [list(range(NCORES))]


def build_body(tc, out, ins):
    nc = tc.nc
    feats = ins["feats_sl"]     # [M, DPC] this core's feature slice (bf16)
    lf = ins["lf"]              # [M, D] local_features pre-transposed to (stripe, d)
    lab = ins["lab"]            # [1, M] labels as f32
    anc = ins["anc"]            # [NB, 1] this core's anchor labels (f32)
    wsel = ins["wsel"]          # [64, NB] +I (rows 0-11) / -I (rows 32-43)
    ident = ins["ident"]        # [128, 128] identity
    esel = ins["esel"]          # [96, 96] fold selection
    aavg = ins["aavg"]          # [M, N] chunk-averaging matrix (1/16 entries)

    NT = M // 128               # 12 sample tiles

    with ExitStack() as ctx:
        const = ctx.enter_context(tc.tile_pool(name="const", bufs=1))
        pers = ctx.enter_context(tc.tile_pool(name="pers", bufs=1))
        dram = ctx.enter_context(tc.tile_pool(name="dram", bufs=1, space="DRAM"))

        # ---- constants ----
        ident_sb = const.tile([128, 128], F32)
        nc.sync.dma_start(ident_sb, ident)
        anc_sb = const.tile([NB, 1], F32)
        nc.sync.dma_start(anc_sb, anc)
        wsel_sb = const.tile([64, NB], F32)
        nc.sync.dma_start(wsel_sb, wsel)
        lab_sb = const.tile([1, M], F32)
        nc.sync.dma_start(lab_sb, lab)
        esel_sb = const.tile([N, N], F32)
        nc.sync.dma_start(esel_sb, esel)
        # averaging matrix tiled [128, 12, 96] (partition p, tile t, center n)
        a_sb = const.tile([128, NT, N], BF16)
        nc.sync.dma_start(a_sb, aavg.rearrange("(t p) n -> p t n", p=128))
        ones_n = const.tile([1, N], F32)
        nc.vector.memset(ones_n, 1.0)
        ones_r = const.tile([1, NB], F32)
        nc.vector.memset(ones_r, 1.0)
        ones_c = const.tile([NB, 1], F32)
        nc.vector.memset(ones_c, 1.0)
        marg = const.tile([NB, 1], F32)
        nc.vector.memset(marg, MARGIN)

        ctr_cm = pers.tile([N, DPC], F32)   # center-major centers slice
        # s_sb and x_all are time-disjoint: share one 32KB/partition slot
        s_sb = pers.tile([N, M], F32)   # partial mining scores

        # DRAM bounce buffers for collectives
        rs_in = dram.tile([N, M], F32)
        rs_out = dram.tile([NB, M], F32)
        a2a_in = dram.tile([N, DPC], F32)
        a2a_out = dram.tile([N, DPC], F32)

        # ---- phase 1: natural-layout loads + on-chip transpose ----
        with tc.tile_pool(name="psumc", bufs=1, space="PSUM") as psumc, \
             tc.tile_pool(name="psums", bufs=1, space="PSUM") as psums, \
             tc.tile_pool(name="psumt", bufs=2, space="PSUM") as psumt, \
             tc.tile_pool(name="psumn", bufs=1, space="PSUM") as psumn, \
             tc.tile_pool(name="ftTp", bufs=1) as ftTp, \
             tc.tile_pool(name="kio", bufs=3) as kio, \
             tc.tile_pool(name="workp1", bufs=2) as work:
            ps_c = [psumc.tile([N, 512], F32, name=f"ps_c{h}") for h in range(2)]
            s_ps = [psums.tile([N, 512], F32, name=f"s_ps{b}") for b in range(3)]
            ftT = [ftTp.tile([128, M], BF16, name=f"ftT{kd}") for kd in range(8)]
            normcol = pers.tile([128, NT], F32)
            # featsT k-tiles via hardware DMA transpose (bf16, xbar)
            for kd in range(8):
                nc.sync.dma_start_transpose(
                    ftT[kd], feats[:, kd * 128:(kd + 1) * 128]
                )
            for t in range(NT):
                nt = kio.tile([128, DPC], BF16, tag="nt")
                nc.sync.dma_start(nt, feats[t * 128:(t + 1) * 128, :])
                # centers: [96, 1024-slice] center-major via averaging matmul
                for h in range(2):
                    nc.tensor.matmul(
                        ps_c[h], lhsT=a_sb[:, t, :],
                        rhs=nt[:, h * 512:(h + 1) * 512],
                        start=(t == 0), stop=(t == NT - 1),
                    )
                # per-sample squared norms for this tile
                sqd = work.tile([128, DPC], F32, tag="sqd")
                nc.scalar.activation(
                    sqd, nt, AF.Square, accum_out=normcol[:, t:t + 1]
                )
            # centers psum -> SBUF (true centers; aavg already has the 1/16)
            for h in range(2):
                nc.vector.tensor_copy(
                    ctr_cm[:, h * 512:(h + 1) * 512], ps_c[h]
                )
            # ctrT blocks scaled by -2 for the dot matmuls
            ctrT2 = []
            for kd in range(8):
                tpc = psumt.tile([128, N], F32, tag="tp")
                nc.tensor.transpose(
                    tpc, ctr_cm[:, kd * 128:(kd + 1) * 128],
                    ident_sb[:N, :N],
                )
                c2 = work.tile([128, N], BF16, tag=f"ctrT{kd}", bufs=1)
                nc.vector.tensor_scalar_mul(c2, tpc, -2.0)
                ctrT2.append(c2)
            # norm row [1, 1536] from normcol via transposes
            nrow = pers.tile([1, M], F32)
            for t in range(NT):
                tpn = psumn.tile([1, 128], F32, tag="tpn")
                nc.tensor.transpose(tpn, normcol[:, t:t + 1], ident_sb)
                nc.vector.tensor_copy(nrow[:, t * 128:(t + 1) * 128], tpn)
            # scores: s = norm[m] - 2 c.f accumulated over 8 k-tiles
            for kd in range(8):
                for b in range(3):
                    nc.tensor.matmul(
                        s_ps[b], lhsT=ctrT2[kd],
                        rhs=ftT[kd][:, b * 512:(b + 1) * 512],
                        start=(kd == 0), stop=False,
                    )
            for b in range(3):
                nc.tensor.matmul(
                    s_ps[b], lhsT=ones_n, rhs=nrow[:, b * 512:(b + 1) * 512],
                    start=False, stop=True,
                )
            for b in range(3):
                nc.vector.tensor_copy(s_sb[:, b * 512:(b + 1) * 512], s_ps[b])

        nc.sync.dma_start(a2a_in, ctr_cm)
        nc.sync.dma_start(rs_in, s_sb)
        nc.gpsimd.collective_compute(
            "AllToAll", ALU.bypass, replica_groups=RG,
            ins=[a2a_in.opt()], outs=[a2a_out.opt()],
        )
        nc.gpsimd.collective_compute(
            "ReduceScatter", ALU.add, replica_groups=RG,
            ins=[rs_in.opt()], outs=[rs_out.opt()],
        )

        # ---- X_all prefetch: [96 (b,j-rep), 8192 (i,d)] from a2a_out ----
        # a2a_out row (i*NB + b) holds center b's features [i*1024, (i+1)*1024)
        x_all = pers.tile([N, D], F32)
        x_v = a2a_out.rearrange("(i b) d -> b i d", b=NB)  # [12, 8, 1024]
        for b in range(NB):
            # center b's full row, replicated over the 8 j-partitions
            nc.sync.dma_start(
                x_all[b * S:(b + 1) * S, :],
                x_v[b:b + 1, :, :].to_broadcast([S, S, DPC]),
            )

        # ---- mining on this core's 12 score rows ----
        ctx2 = tc.tile_pool(name="mine", bufs=4)
        mine = ctx2.__enter__()
        s12 = mine.tile([NB, M], F32, tag="mine", name="s12")
        nc.sync.dma_start(s12, rs_out)
        diff = mine.tile([NB, M], F32, tag="mine", name="diff")
        m01 = mine.tile([NB, M], F32, tag="mine", name="m01")
        m01c = mine.tile([NB, M], F32, tag="mine", name="m01c")
        pos = mine.tile([NB, M], F32, tag="mine", name="pos")
        neg = mine.tile([NB, M], F32, tag="mine", name="neg")
        with tc.tile_pool(name="psum2", bufs=1, space="PSUM") as psum2:
            lbc = psum2.tile([NB, M], F32)
            for b in range(3):
                nc.tensor.matmul(
                    lbc[:, b * 512:(b + 1) * 512], lhsT=ones_r,
                    rhs=lab_sb[:, b * 512:(b + 1) * 512], start=True, stop=True,
                )
            # diff = L[m] - A[b] per partition
            nc.vector.tensor_scalar(
                out=diff, in0=lbc, scalar1=anc_sb, scalar2=None, op0=ALU.subtract
            )
        nc.vector.tensor_tensor(out=m01, in0=diff, in1=diff, op=ALU.mult)
        # pen = BIGM iff labels differ else 0 (constant row shifts don't
        # change per-row argmax, so no -BIGM offset needed)
        nc.vector.tensor_scalar(
            out=m01c, in0=m01, scalar1=1.0, scalar2=BIGM,
            op0=ALU.min, op1=ALU.mult,
        )
        nc.vector.tensor_tensor(out=pos, in0=s12, in1=m01c, op=ALU.subtract)
        nc.vector.tensor_tensor(out=neg, in0=m01c, in1=s12, op=ALU.subtract)

        pmax = pers.tile([NB, 8], F32, tag="pmax")
        pidx = pers.tile([NB, 8], U32)
        nmax = pers.tile([NB, 8], F32, tag="nmax")
        nidx = pers.tile([NB, 8], U32)
        nc.vector.max(pmax, pos)
        nc.vector.max_index(pidx, pmax, pos)
        nc.vector.max(nmax, neg)
        nc.vector.max_index(nidx, nmax, neg)
        ctx2.__exit__(None, None, None)

        # ---- phase 2: gather winners + local distances ----
        # tanh'd dists, padded 9x9 grid; p-set at partitions 0-11, n-set at
        # 32-43 (compute ops need 32-aligned start partitions)
        dD = pers.tile([64, 81], F32)
        nc.vector.memset(dD, 0.0)
        with tc.tile_pool(name="yrawp", bufs=2) as yrawp, \
             tc.tile_pool(name="workp2", bufs=2) as work2:
            for iset, idx in enumerate((pidx, nidx)):
                raw = yrawp.tile([NB, D], F32, tag="yraw", name=f"yraw{iset}")
                nc.gpsimd.indirect_dma_start(
                    out=raw, out_offset=None, in_=lf,
                    in_offset=bass.IndirectOffsetOnAxis(ap=idx[:, :1], axis=0),
                )
                # fold to 96 partitions: [12, 8192] -> [96 (b,j), 1024 (d)]
                y96 = work2.tile([N, DL], F32, tag="y96")
                nc.sync.dma_start(y96, raw)
                d2t = work2.tile([N, S], F32, tag="d2t")
                for i in range(S):
                    dtmp = work2.tile([N, DL], F32, tag="dtmp")
                    nc.vector.tensor_tensor(
                        out=dtmp, in0=y96, in1=x_all[:, i * DL:(i + 1) * DL],
                        op=ALU.subtract,
                    )
                    dsq = work2.tile([N, DL], F32, tag="dsq")
                    nc.scalar.activation(
                        dsq, dtmp, AF.Square, accum_out=d2t[:, i:i + 1]
                    )
                dcl = work2.tile([N, S], F32, tag="dcl")
                nc.vector.tensor_scalar_max(dcl, d2t, 1e-12)
                dsqr = work2.tile([N, S], F32, tag="dsqr")
                nc.scalar.activation(dsqr, dcl, AF.Sqrt)
                dsg = work2.tile([N, S], F32, tag="dsg")
                nc.scalar.activation(dsg, dsqr, AF.Tanh, scale=0.5)
                # fold [96 (b,j), 8 (i)] -> dD[b + iset*NB, 9*i + j + 1] via PE:
                # for each j: [12, 8(i)] block = esel[:, j-block].T @ dsg
                with tc.tile_pool(name=f"psf{iset}", bufs=1, space="PSUM") as psf_p:
                    psf = psf_p.tile([NB, 64], F32)
                    for j in range(S):
                        nc.tensor.matmul(
                            psf[:, j * S:(j + 1) * S],
                            lhsT=esel_sb[:, j * NB:(j + 1) * NB],
                            rhs=dsg, start=True, stop=True,
                        )
                    dst = dD[iset * 32:iset * 32 + NB, 0:72].rearrange(
                        "p (i j) -> p j i", j=9
                    )[:, 1:9, :]
                    nc.vector.tensor_copy(
                        dst, psf.rearrange("p (j i) -> p j i", i=S)
                    )

        # ---- shortest-path DP on anti-diagonals (9x9 padded grid) ----
        dp = pers.tile([64, 81], F32)
        nc.vector.memset(dp, BIGDP)
        nc.vector.memset(dp[:, 1:2], 0.0)
        for kdiag in range(2, 17):
            lo = max(1, kdiag - 8)
            hi = min(8, kdiag - 1)
            cnt = hi - lo + 1
            f0 = 9 * lo + (kdiag - lo)
            t = pers.tile([64, 8], F32, tag="dptmp", bufs=2)
            nc.vector.tensor_tensor(
                out=t[:, :cnt],
                in0=dp[:, f0 - 9:f0 - 9 + 8 * (cnt - 1) + 1:8],
                in1=dp[:, f0 - 1:f0 - 1 + 8 * (cnt - 1) + 1:8],
                op=ALU.min,
            )
            nc.vector.tensor_tensor(
                out=dp[:, f0:f0 + 8 * (cnt - 1) + 1:8],
                in0=t[:, :cnt],
                in1=dD[:, f0 - 9:f0 - 9 + 8 * (cnt - 1) + 1:8],
                op=ALU.add,
            )

        # ---- loss: relu(ap - an + margin), partial sum over 12 centers ----
        with tc.tile_pool(name="psum3", bufs=1, space="PSUM") as psum3:
            dps = psum3.tile([NB, 1], F32)
            nc.tensor.matmul(dps, lhsT=wsel_sb, rhs=dp[:, 80:81], start=True, stop=True)
            r12 = pers.tile([NB, 1], F32)
            nc.scalar.activation(r12, dps, AF.Relu, bias=marg)
            lsum = psum3.tile([1, 1], F32)
            nc.tensor.matmul(lsum, lhsT=r12, rhs=ones_c, start=True, stop=True)
            out_sb = pers.tile([1, 1], F32)
            nc.vector.tensor_copy(out_sb, lsum)
        nc.sync.dma_start(out, out_sb)


def build_program():
    nc = bacc.Bacc(
        "TRN2", target_bir_lowering=False, debug=False,
        enable_asserts=False, num_devices=NCORES,
    )
    ins = {
        "feats_sl": nc.dram_tensor("feats_sl", [M, DPC], BF16, kind="ExternalInput").ap(),
        "lf": nc.dram_tensor("lf", [M, D], F32, kind="ExternalInput").ap(),
        "lab": nc.dram_tensor("lab", [1, M], F32, kind="ExternalInput").ap(),
        "anc": nc.dram_tensor("anc", [NB, 1], F32, kind="ExternalInput").ap(),
        "wsel": nc.dram_tensor("wsel", [64, NB], F32, kind="ExternalInput").ap(),
        "ident": nc.dram_tensor("ident", [128, 128], F32, kind="ExternalInput").ap(),
        "esel": nc.dram_tensor("esel", [N, N], F32, kind="ExternalInput").ap(),
        "aavg": nc.dram_tensor("aavg", [M, N], BF16, kind="ExternalInput").ap(),
    }
    out = nc.dram_tensor("out", [1, 1], F32, kind="ExternalOutput").ap()
    with tile.TileContext(nc) as tc:
        build_body(tc, out, ins)
    nc.compile()
    return nc


def make_in_maps(feats, labels, local_features):
    bf16 = mybir.dt.np(BF16)
    feats = np.asarray(feats, dtype=np.float32).astype(bf16)
    labf = np.asarray(labels).astype(np.float32)
    lf_flat = np.ascontiguousarray(
        np.asarray(local_features, dtype=np.float32).transpose(0, 2, 1).reshape(M, D)
    )
    lab_row = labf.reshape(1, M)
    anchors = labf[::K]  # [96]
    wsel = np.zeros((64, NB), dtype=np.float32)
    wsel[0:NB, :] = np.eye(NB)
    wsel[32:32 + NB, :] = -np.eye(NB)
    ident = np.eye(128, dtype=np.float32)
    esel = np.zeros((N, N), dtype=np.float32)
    for b in range(NB):
        for j in range(S):
            esel[b * S + j, j * NB + b] = 1.0
    aavg = np.zeros((M, N), dtype=np.float32)
    aavg[np.arange(M), np.arange(M) // K] = 1.0 / K
    aavg = aavg.astype(bf16)
    in_maps = []
    for c in range(NCORES):
        in_maps.append({
            "feats_sl": np.ascontiguousarray(feats[:, c * DPC:(c + 1) * DPC]),
            "lf": lf_flat,
            "lab": lab_row,
            "anc": np.ascontiguousarray(
                anchors[c * NB:(c + 1) * NB].reshape(NB, 1)
            ),
            "wsel": wsel,
            "ident": ident,
            "esel": esel,
            "aavg": aavg,
        })
    return in_maps


_NC_CACHE = None


def _get_nc():
    global _NC_CACHE
    if _NC_CACHE is None:
        _NC_CACHE = build_program()
    return _NC_CACHE


def run(feats, labels, local_features, trace=False, **kwargs):
    nc = _get_nc()
    in_maps = make_in_maps(feats, labels, local_features)
    res = bass_utils.run_bass_kernel_spmd(
        nc, in_maps, core_ids=list(range(NCORES)), trace=trace, **kwargs
    )
    partial = sum(float(r["out"][0, 0]) for r in res.results)
    return np.float32(partial / N), res


def kernel(feats, labels, local_features):
    loss, _ = run(feats, labels, local_features)
    return loss
